# revision 6
# baseline (speedup 1.0000x reference)
"""Trainium2 Bass kernel for nn_ContrastiveLoss_rec (8-core data-parallel).

Math (per reference):
    wA_is = A_is @ W.T + b ; wA_em = A_em @ W.T + b
    diag_is = sum((0.4*m + 0.6*tr_m) * wA_is, -1)
    diag_em = sum((0.4*m + 0.6*tr_m) * wA_em, -1)
    loss = sum(max(0.2 + diag_is - diag_em, 0))

Algebraic simplification used here:
    mc  = 0.4*m + 0.6*tr_m          (bias b cancels in the difference)
    z   = rowdot(mc, (A_is - A_em) @ W.T)
        = rowdot(D, mc @ W)          with D = A_is - A_em
    loss = sum(max(0.2 + z, 0))
Folding the 0.6:  mc = 0.6*(tr_m + (2/3) m) = 0.6*mc'
    loss = 0.6 * sum(max(z' + 1/3, 0)),  z' = rowdot(D, mc' @ W)

Per-core plan (B_loc = 1024 rows):
  - Inputs split across BOTH HWDGE rings: sync carries m + A_is + even W
    chunks, scalar (Activation) carries tr_m + A_em + odd W chunks.
  - W declared fp32r in DRAM (same bits as fp32), loaded as 8 chunk tiles,
    then cast on the Activation engine into fp8e4 pair tiles [128, 2, E]
    laid out for DoubleRow matmuls.
  - mc' = (2/3)m + tr_m in ONE DVE scalar_tensor_tensor, cast to fp8e4 on
    write. PE transposes the fp8 chunks (1 cycle/row) via an fp8 identity;
    Activation drains the PSUM transposes to SBUF.
  - Main matmul in fp8e4 DoubleRow perf mode (2 k-tiles of 128 per
    instruction, 2x bf16 rate): P = mc' @ W accumulated in fp32 PSUM.
  - DVE fused tensor_tensor_reduce in fp32: z'_part = sum(D * P, free).
  - Hinge + row reduce, partition reduce via matmul with a 0.6-filled ones
    vector, scalar out per core; host sums the 8 partials.
Only mc' and W are quantized to fp8 (D and the rowdot stay fp32):
measured end-to-end rel err ~6e-4 against the fp32 reference.
"""

import numpy as np

import concourse.bass as bass
import concourse.mybir as mybir
import concourse.tile as tile
from concourse.bass_utils import run_bass_kernel_spmd

N_CORES = 8
B, E = 8192, 1024
B_LOC = B // N_CORES          # 1024 rows per core
P = 128                       # partitions
NBT = B_LOC // P              # 8 b-tiles per core
ST = 2                        # b-tiles per DMA super-tile (1 MiB DMAs)
KT = E // P                   # 8 contraction chunks
KP = KT // 2                  # 4 DoubleRow k-pairs
NF = 256                      # DR moving free dim (rhs free = 2*NF = 512)
NCH = E // NF                 # 4 n-chunks

F32 = mybir.dt.float32
F32R = mybir.dt.float32r
FP8 = mybir.dt.float8e4
AX = mybir.AluOpType
DR = mybir.MatmulPerfMode.DoubleRow


def _make_scaled_identity(nc, ap, val):
    nc.gpsimd.memset(ap, 0.0)
    nc.gpsimd.affine_select(
        out=ap,
        in_=ap,
        compare_op=AX.not_equal,
        fill=float(val),
        base=0,
        pattern=[[-1, ap.shape[1]]],
        channel_multiplier=1,
    )


def build(st=ST, io_bufs=2, w_bufs=2, repeat=1):
    """Build the single-core Bass program (SPMD across 8 cores)."""
    nst = NBT // st
    nc = bass.Bass(
        "TRN2", target_bir_lowering=False, debug=False, num_devices=N_CORES
    )

    A_is = nc.dram_tensor("a_is", [B_LOC, E], F32, kind="ExternalInput").ap()
    A_em = nc.dram_tensor("a_em", [B_LOC, E], F32, kind="ExternalInput").ap()
    M_in = nc.dram_tensor("m_in", [B_LOC, E], F32, kind="ExternalInput").ap()
    TR_m = nc.dram_tensor("tr_m", [B_LOC, E], F32, kind="ExternalInput").ap()
    # fp32r has identical bits to fp32: declaring the DRAM tensor fp32r
    # lets the HWDGE rings load it with no SWDGE cast DMA.
    W_in = nc.dram_tensor("w_in", [E, E], F32R, kind="ExternalInput").ap()
    OUT = nc.dram_tensor("out", [1, 1], F32, kind="ExternalOutput").ap()

    with tile.TileContext(nc) as tc:
        with (
            tc.tile_pool(name="const", bufs=1) as cpool,
            tc.tile_pool(name="wpool", bufs=w_bufs) as wpool,
            tc.tile_pool(name="w8pool", bufs=w_bufs) as w8pool,
            tc.tile_pool(name="io", bufs=io_bufs) as iopool,
            tc.tile_pool(name="dbuf", bufs=2) as dpool,
            tc.tile_pool(name="mct", bufs=2) as mctpool,
            tc.tile_pool(name="ttr", bufs=2) as ttrpool,
            tc.tile_pool(name="acc", bufs=2) as accpool,
            tc.tile_pool(name="ps_t", bufs=4, space="PSUM") as pst,
            tc.tile_pool(name="ps_mm", bufs=2, space="PSUM") as psmm,
            tc.tile_pool(name="ps_fin", bufs=1, space="PSUM") as psfin,
        ):
            ident_f32 = cpool.tile([P, P], F32)
            _make_scaled_identity(nc, ident_f32[:], 1.0)
            ident8 = cpool.tile([P, P], FP8)
            nc.vector.tensor_copy(ident8[:], ident_f32[:])
            ones06 = cpool.tile([P, 1], F32)
            nc.vector.memset(ones06[:], 0.6)

            for _rep in range(repeat):
                # z' partials: one column per (b-tile, n-chunk)
                zacc = accpool.tile([P, NBT * NCH], F32, tag="zacc")

                # Super-tile 0 inputs first so compute can start early,
                # then the 8 W chunks (even->sync, odd->scalar), then the
                # rest. Ring FIFO order == consumption order.
                m_ts, trm_ts, ais_ts, aem_ts = [], [], [], []

                def load_st(s):
                    rows = bass.ds(s * st * P, st * P)
                    m_t = iopool.tile([P, st, E], F32, tag="m", name=f"m{s}")
                    trm_t = iopool.tile([P, st, E], F32, tag="trm", name=f"trm{s}")
                    nc.sync.dma_start(
                        m_t[:], M_in[rows, :].rearrange("(t p) e -> p t e", p=P)
                    )
                    nc.scalar.dma_start(
                        trm_t[:], TR_m[rows, :].rearrange("(t p) e -> p t e", p=P)
                    )
                    m_ts.append(m_t)
                    trm_ts.append(trm_t)

                def load_st2(s):
                    rows = bass.ds(s * st * P, st * P)
                    ais_t = iopool.tile([P, st, E], F32, tag="ais", name=f"ais{s}")
                    aem_t = iopool.tile([P, st, E], F32, tag="aem", name=f"aem{s}")
                    nc.sync.dma_start(
                        ais_t[:], A_is[rows, :].rearrange("(t p) e -> p t e", p=P)
                    )
                    nc.scalar.dma_start(
                        aem_t[:], A_em[rows, :].rearrange("(t p) e -> p t e", p=P)
                    )
                    ais_ts.append(ais_t)
                    aem_ts.append(aem_t)

                load_st(0)
                w_sb = []
                for k in range(KT):
                    wk = wpool.tile([P, E], F32R, tag=f"w{k}", name=f"w{k}")
                    eng = nc.sync if k % 2 == 0 else nc.scalar
                    eng.dma_start(wk[:], W_in[bass.ds(k * P, P), :])
                    w_sb.append(wk)
                load_st2(0)
                for s in range(1, nst):
                    load_st(s)
                    load_st2(s)

                # fp8 DoubleRow W pair tiles: w8[g][p, i, n] = W[(2g+i)*128+p, n]
                w8 = []
                for g in range(KP):
                    w8g = w8pool.tile([P, 2, E], FP8, tag=f"w8{g}", name=f"w8{g}")
                    for i in range(2):
                        nc.scalar.copy(w8g[:, i], w_sb[2 * g + i][:])
                    w8.append(w8g)

                for s in range(nst):
                    # mc' = (2/3)*m + tr_m in one DVE op, fp8e4 out
                    mc_t = dpool.tile([P, st, E], FP8, tag="mc")
                    nc.vector.scalar_tensor_tensor(
                        out=mc_t[:],
                        in0=m_ts[s][:],
                        scalar=2.0 / 3.0,
                        in1=trm_ts[s][:],
                        op0=AX.mult,
                        op1=AX.add,
                    )
                    # D = A_is - A_em  (natural layout, fp32)
                    d_t = dpool.tile([P, st, E], F32, tag="d")
                    nc.vector.tensor_tensor(
                        d_t[:], ais_ts[s][:], aem_ts[s][:], AX.subtract
                    )

                    for t in range(st):
                        # mc'^T chunks via fp8 PE transpose (1 cyc/row);
                        # PSUM->SBUF drain on the Activation engine.
                        mct_t = mctpool.tile([P, KT, P], FP8, tag="mct")
                        for g in range(KT // 4):
                            # fp8 transpose writes PSUM with element step 2:
                            # expose the stride via a trailing dim of 2.
                            pt = pst.tile([P, 4, P, 2], FP8, tag="pt")
                            for j4 in range(4):
                                j = g * 4 + j4
                                cols = bass.ds(j * P, P)
                                nc.tensor.matmul(
                                    pt[:, j4, :, 0],
                                    mc_t[:, t, cols],
                                    ident8[:],
                                    is_transpose=True,
                                    start=True,
                                    stop=True,
                                )
                            nc.scalar.copy(
                                mct_t[:, bass.ds(g * 4, 4), :], pt[:, :, :, 0]
                            )

                        # P = mc' @ W in fp8 DoubleRow (contraction 256/instr)
                        for n in range(NCH):
                            ncols = bass.ds(n * NF, NF)
                            pm = psmm.tile([P, NF], F32, tag="pm")
                            for g in range(KP):
                                nc.tensor.matmul(
                                    pm[:],
                                    mct_t[:, bass.ds(2 * g, 2), :],
                                    w8[g][:, :, ncols],
                                    start=(g == 0),
                                    stop=(g == KP - 1),
                                    perf_mode=DR,
                                )
                            ttr_out = ttrpool.tile([P, NF], F32, tag="ttro")
                            zi = (s * st + t) * NCH + n
                            nc.vector.scalar_tensor_tensor(
                                out=ttr_out[:],
                                in0=pm[:],
                                scalar=1.0,
                                in1=d_t[:, t, ncols],
                                op0=AX.mult,
                                op1=AX.mult,
                                accum_out=zacc[:, zi : zi + 1],
                            )

                # z'_b = sum of its 4 n-chunk partials; hinge; row-reduce
                zv = zacc[:].rearrange("p (b n) -> p b n", n=NCH)
                zp = accpool.tile([P, NBT, 2], F32, tag="zp")
                nc.vector.tensor_tensor(
                    zp[:, :, 0], zv[:, :, 0], zv[:, :, 1], AX.add
                )
                nc.vector.tensor_tensor(
                    zp[:, :, 1], zv[:, :, 2], zv[:, :, 3], AX.add
                )
                zrow = accpool.tile([P, NBT], F32, tag="zrow")
                nc.vector.tensor_tensor(
                    zrow[:], zp[:, :, 0], zp[:, :, 1], AX.add
                )
                hrow = accpool.tile([P, NBT], F32, tag="hrow")
                nc.vector.tensor_scalar(
                    hrow[:], zrow[:], 1.0 / 3.0, 0.0, AX.add, AX.max
                )
                hsum = accpool.tile([P, 1], F32, tag="hsum")
                nc.vector.reduce_sum(hsum[:], hrow[:], axis=mybir.AxisListType.X)

                # partition reduce (x0.6 folded into the ones vector)
                fin = psfin.tile([1, 1], F32, tag="fin")
                nc.tensor.matmul(fin[:], hsum[:], ones06[:], start=True, stop=True)
                out_sb = accpool.tile([1, 1], F32, tag="osb")
                nc.any.tensor_copy(out_sb[:], fin[:])
                nc.sync.dma_start(OUT[:], out_sb[:])

    return nc


def _split_multi_waits(raw: bytes) -> bytes:
    """Split multi-wait instructions into single-wait Drain carriers +
    original: this walrus build allows only one sync wait per instruction."""
    import json as _json

    d = _json.loads(raw)
    for fn in d["functions"]:
        for bb in fn["blocks"]:
            out = []
            for inst in bb["instructions"]:
                si = inst.get("sync_info") or {}
                waits = si.get("on_wait") or []
                if len(waits) > 1:
                    for i, w in enumerate(waits[:-1]):
                        carrier = {
                            "engine": inst["engine"],
                            "ins": [],
                            "name": f"{inst['name']}-sw{i}",
                            "opcode": "Drain",
                            "outs": [],
                            "sync_info": {"on_update": [], "on_wait": [w]},
                        }
                        if "debug" in inst:
                            carrier["debug"] = inst["debug"]
                        out.append(carrier)
                    inst["sync_info"] = {
                        "on_update": si.get("on_update") or [],
                        "on_wait": [waits[-1]],
                    }
                out.append(inst)
            bb["instructions"] = out
    return _json.dumps(d).encode()


def _patch_nc(nc):
    patched = _split_multi_waits(nc.to_json_bytes())
    nc.to_json_bytes = lambda: patched
    return nc


_NC_CACHE = None


def _get_nc():
    global _NC_CACHE
    if _NC_CACHE is None:
        _NC_CACHE = _patch_nc(build())
    return _NC_CACHE


def _in_maps(inputs):
    a_is = np.ascontiguousarray(np.asarray(inputs["A_is_t"], dtype=np.float32))
    a_em = np.ascontiguousarray(np.asarray(inputs["A_em_t"], dtype=np.float32))
    m = np.ascontiguousarray(np.asarray(inputs["m"], dtype=np.float32))
    tr_m = np.ascontiguousarray(np.asarray(inputs["tr_m"], dtype=np.float32))
    w = np.ascontiguousarray(np.asarray(inputs["W"], dtype=np.float32))
    maps = []
    for c in range(N_CORES):
        sl = slice(c * B_LOC, (c + 1) * B_LOC)
        maps.append(
            {
                "a_is": a_is[sl],
                "a_em": a_em[sl],
                "m_in": m[sl],
                "tr_m": tr_m[sl],
                "w_in": w,
            }
        )
    return maps


def run(inputs, trace=False, **kw):
    """Run on all 8 cores; returns (full_output, BassKernelResults)."""
    nc = _get_nc()
    res = run_bass_kernel_spmd(
        nc, _in_maps(inputs), list(range(N_CORES)), trace=trace, **kw
    )
    total = float(sum(np.float32(r["out"][0, 0]) for r in res.results))
    return np.array([total], dtype=np.float32), res


def kernel(**inputs) -> np.ndarray:
    out, _ = run(inputs, trace=False)
    return out


# revision 7
# speedup vs baseline: 2.6976x; 2.6976x over previous
"""Trainium2 Bass kernel for nn_ContrastiveLoss_rec (8-core data-parallel).

Math (per reference):
    wA_is = A_is @ W.T + b ; wA_em = A_em @ W.T + b
    diag_is = sum((0.4*m + 0.6*tr_m) * wA_is, -1)
    diag_em = sum((0.4*m + 0.6*tr_m) * wA_em, -1)
    loss = sum(max(0.2 + diag_is - diag_em, 0))

Algebraic simplification used here:
    mc  = 0.4*m + 0.6*tr_m          (bias b cancels in the difference)
    z   = rowdot(mc, (A_is - A_em) @ W.T)
        = rowdot(D, mc @ W)          with D = A_is - A_em
    loss = sum(max(0.2 + z, 0))
Folding the 0.6:  mc = 0.6*(tr_m + (2/3) m) = 0.6*mc'
    loss = 0.6 * sum(max(z' + 1/3, 0)),  z' = rowdot(D, mc' @ W)

Per-core plan (B_loc = 1024 rows):
  - Inputs split across BOTH HWDGE rings: sync carries m + A_is + even W
    chunks, scalar (Activation) carries tr_m + A_em + odd W chunks.
  - W is declared fp32r in DRAM (same bits as fp32) and loaded as 8
    independent 128-row chunk tiles so matmuls wait per-chunk, not on a
    monolithic 4 MiB load.
  - mc' = (2/3)m + tr_m in ONE DVE scalar_tensor_tensor, output fp32r.
  - PE transposes mc' chunks (fp32r, 1.5 cycles/row); PSUM->SBUF copies of
    the transposed chunks run on the Activation engine to keep DVE free.
  - Main matmul P = mc' @ W in float32r (full-rate fp32 path).
  - DVE fused tensor_tensor_reduce: z'_partial = sum(D * P, free-axis).
  - Hinge + row reduce, partition reduce via matmul with a 0.6-filled ones
    vector, scalar out per core; host sums the 8 partials.
"""

import numpy as np

import concourse.bass as bass
import concourse.mybir as mybir
import concourse.tile as tile
from concourse.bass_utils import run_bass_kernel_spmd

N_CORES = 8
B, E = 8192, 1024
B_LOC = B // N_CORES          # 1024 rows per core
P = 128                       # partitions
NBT = B_LOC // P              # 8 b-tiles per core
ST = 2                        # b-tiles per DMA super-tile (1 MiB DMAs)
KT = E // P                   # 8 contraction chunks
NF = 512                      # matmul moving free dim (one PSUM bank fp32)
NCH = E // NF                 # 2 n-chunks

F32 = mybir.dt.float32
F32R = mybir.dt.float32r
BF16 = mybir.dt.bfloat16
AX = mybir.AluOpType


def _make_scaled_identity(nc, ap, val):
    nc.gpsimd.memset(ap, 0.0)
    nc.gpsimd.affine_select(
        out=ap,
        in_=ap,
        compare_op=AX.not_equal,
        fill=float(val),
        base=0,
        pattern=[[-1, ap.shape[1]]],
        channel_multiplier=1,
    )


def build(st=ST, io_bufs=2, w_bufs=2, repeat=1, bf16_t=False):
    """Build the single-core Bass program (SPMD across 8 cores).

    bf16_t: transpose mc' in bf16 (1 cyc/row on PE instead of 1.5); the
    Activation-engine PSUM drain casts back to fp32r for the main matmul.
    """
    nst = NBT // st
    tdt = BF16 if bf16_t else F32R
    nc = bass.Bass(
        "TRN2", target_bir_lowering=False, debug=False, num_devices=N_CORES
    )

    A_is = nc.dram_tensor("a_is", [B_LOC, E], F32, kind="ExternalInput").ap()
    A_em = nc.dram_tensor("a_em", [B_LOC, E], F32, kind="ExternalInput").ap()
    M_in = nc.dram_tensor("m_in", [B_LOC, E], F32, kind="ExternalInput").ap()
    TR_m = nc.dram_tensor("tr_m", [B_LOC, E], F32, kind="ExternalInput").ap()
    # fp32r has identical bits to fp32: declaring the DRAM tensor fp32r
    # lets the HWDGE rings load it with no SWDGE cast DMA.
    W_in = nc.dram_tensor("w_in", [E, E], F32R, kind="ExternalInput").ap()
    OUT = nc.dram_tensor("out", [1, 1], F32, kind="ExternalOutput").ap()

    with tile.TileContext(nc) as tc:
        with (
            tc.tile_pool(name="const", bufs=1) as cpool,
            tc.tile_pool(name="wpool", bufs=w_bufs) as wpool,
            tc.tile_pool(name="io", bufs=io_bufs) as iopool,
            tc.tile_pool(name="dbuf", bufs=2) as dpool,
            tc.tile_pool(name="mct", bufs=2) as mctpool,
            tc.tile_pool(name="ttr", bufs=2) as ttrpool,
            tc.tile_pool(name="acc", bufs=2) as accpool,
            tc.tile_pool(name="ps_t", bufs=4, space="PSUM") as pst,
            tc.tile_pool(name="ps_mm", bufs=2, space="PSUM") as psmm,
            tc.tile_pool(name="ps_fin", bufs=1, space="PSUM") as psfin,
        ):
            ident_f32 = cpool.tile([P, P], F32)
            _make_scaled_identity(nc, ident_f32[:], 1.0)
            ident1 = cpool.tile([P, P], tdt)
            nc.vector.tensor_copy(ident1[:], ident_f32[:])
            ones06 = cpool.tile([P, 1], F32)
            nc.vector.memset(ones06[:], 0.6)

            for _rep in range(repeat):
                # z' partials: one column per (b-tile, n-chunk)
                zacc = accpool.tile([P, NBT * NCH], F32, tag="zacc")

                # Super-tile 0 inputs first so compute can start early,
                # then the 8 W chunks (even->sync, odd->scalar), then the
                # rest. Ring FIFO order == consumption order.
                m_ts, trm_ts, ais_ts, aem_ts = [], [], [], []

                def load_st(s):
                    rows = bass.ds(s * st * P, st * P)
                    m_t = iopool.tile([P, st, E], F32, tag="m", name=f"m{s}")
                    trm_t = iopool.tile([P, st, E], F32, tag="trm", name=f"trm{s}")
                    nc.sync.dma_start(
                        m_t[:], M_in[rows, :].rearrange("(t p) e -> p t e", p=P)
                    )
                    nc.scalar.dma_start(
                        trm_t[:], TR_m[rows, :].rearrange("(t p) e -> p t e", p=P)
                    )
                    m_ts.append(m_t)
                    trm_ts.append(trm_t)

                def load_st2(s):
                    rows = bass.ds(s * st * P, st * P)
                    ais_t = iopool.tile([P, st, E], F32, tag="ais", name=f"ais{s}")
                    aem_t = iopool.tile([P, st, E], F32, tag="aem", name=f"aem{s}")
                    nc.sync.dma_start(
                        ais_t[:], A_is[rows, :].rearrange("(t p) e -> p t e", p=P)
                    )
                    nc.scalar.dma_start(
                        aem_t[:], A_em[rows, :].rearrange("(t p) e -> p t e", p=P)
                    )
                    ais_ts.append(ais_t)
                    aem_ts.append(aem_t)

                load_st(0)
                w_sb = []
                for k in range(KT):
                    wk = wpool.tile([P, E], F32R, tag=f"w{k}", name=f"w{k}")
                    eng = nc.sync if k % 2 == 0 else nc.scalar
                    eng.dma_start(wk[:], W_in[bass.ds(k * P, P), :])
                    w_sb.append(wk)
                load_st2(0)
                for s in range(1, nst):
                    load_st(s)
                    load_st2(s)

                for s in range(nst):
                    # mc' = (2/3)*m + tr_m in one DVE op
                    mc_t = dpool.tile([P, st, E], tdt, tag="mc")
                    nc.vector.scalar_tensor_tensor(
                        out=mc_t[:],
                        in0=m_ts[s][:],
                        scalar=2.0 / 3.0,
                        in1=trm_ts[s][:],
                        op0=AX.mult,
                        op1=AX.add,
                    )
                    # D = A_is - A_em  (natural layout)
                    d_t = dpool.tile([P, st, E], F32, tag="d")
                    nc.vector.tensor_tensor(
                        d_t[:], ais_ts[s][:], aem_ts[s][:], AX.subtract
                    )

                    for t in range(st):
                        # mc'^T chunks via PE transpose; Activation drains
                        # PSUM->SBUF (casting to fp32r when bf16_t).
                        mct_t = mctpool.tile([P, KT, P], F32R, tag="mct")
                        for g in range(KT // 4):
                            pt = pst.tile([P, 4, P], tdt, tag="pt")
                            for j4 in range(4):
                                j = g * 4 + j4
                                cols = bass.ds(j * P, P)
                                nc.tensor.matmul(
                                    pt[:, j4],
                                    mc_t[:, t, cols],
                                    ident1[:],
                                    is_transpose=True,
                                    start=True,
                                    stop=True,
                                )
                            nc.scalar.copy(mct_t[:, bass.ds(g * 4, 4), :], pt[:])

                        # P = mc' @ W  (float32r full-rate), then fused rowdot
                        for n in range(NCH):
                            ncols = bass.ds(n * NF, NF)
                            pm = psmm.tile([P, NF], F32, tag="pm")
                            for k in range(KT):
                                nc.tensor.matmul(
                                    pm[:],
                                    mct_t[:, k, :],
                                    w_sb[k][:, ncols],
                                    start=(k == 0),
                                    stop=(k == KT - 1),
                                )
                            ttr_out = ttrpool.tile([P, NF], F32, tag="ttro")
                            zi = (s * st + t) * NCH + n
                            nc.vector.scalar_tensor_tensor(
                                out=ttr_out[:],
                                in0=pm[:],
                                scalar=1.0,
                                in1=d_t[:, t, ncols],
                                op0=AX.mult,
                                op1=AX.mult,
                                accum_out=zacc[:, zi : zi + 1],
                            )

                # z'_b = sum of its n-chunk partials; hinge; row-reduce
                zrow = accpool.tile([P, NBT], F32, tag="zrow")
                nc.vector.tensor_tensor(
                    zrow[:],
                    zacc[:].rearrange("p (b n) -> p b n", n=NCH)[:, :, 0],
                    zacc[:].rearrange("p (b n) -> p b n", n=NCH)[:, :, 1],
                    AX.add,
                )
                hrow = accpool.tile([P, NBT], F32, tag="hrow")
                nc.vector.tensor_scalar(
                    hrow[:], zrow[:], 1.0 / 3.0, 0.0, AX.add, AX.max
                )
                hsum = accpool.tile([P, 1], F32, tag="hsum")
                nc.vector.reduce_sum(hsum[:], hrow[:], axis=mybir.AxisListType.X)

                # partition reduce (x0.6 folded into the ones vector)
                fin = psfin.tile([1, 1], F32, tag="fin")
                nc.tensor.matmul(fin[:], hsum[:], ones06[:], start=True, stop=True)
                out_sb = accpool.tile([1, 1], F32, tag="osb")
                nc.any.tensor_copy(out_sb[:], fin[:])
                nc.sync.dma_start(OUT[:], out_sb[:])

    return nc


def _split_multi_waits(raw: bytes) -> bytes:
    """Split multi-wait instructions into single-wait Drain carriers +
    original: this walrus build allows only one sync wait per instruction."""
    import json as _json

    d = _json.loads(raw)
    for fn in d["functions"]:
        for bb in fn["blocks"]:
            out = []
            for inst in bb["instructions"]:
                si = inst.get("sync_info") or {}
                waits = si.get("on_wait") or []
                if len(waits) > 1:
                    for i, w in enumerate(waits[:-1]):
                        carrier = {
                            "engine": inst["engine"],
                            "ins": [],
                            "name": f"{inst['name']}-sw{i}",
                            "opcode": "Drain",
                            "outs": [],
                            "sync_info": {"on_update": [], "on_wait": [w]},
                        }
                        if "debug" in inst:
                            carrier["debug"] = inst["debug"]
                        out.append(carrier)
                    inst["sync_info"] = {
                        "on_update": si.get("on_update") or [],
                        "on_wait": [waits[-1]],
                    }
                out.append(inst)
            bb["instructions"] = out
    return _json.dumps(d).encode()


def _patch_nc(nc):
    patched = _split_multi_waits(nc.to_json_bytes())
    nc.to_json_bytes = lambda: patched
    return nc


_NC_CACHE = None


def _get_nc():
    global _NC_CACHE
    if _NC_CACHE is None:
        _NC_CACHE = _patch_nc(build())
    return _NC_CACHE


def _in_maps(inputs):
    a_is = np.ascontiguousarray(np.asarray(inputs["A_is_t"], dtype=np.float32))
    a_em = np.ascontiguousarray(np.asarray(inputs["A_em_t"], dtype=np.float32))
    m = np.ascontiguousarray(np.asarray(inputs["m"], dtype=np.float32))
    tr_m = np.ascontiguousarray(np.asarray(inputs["tr_m"], dtype=np.float32))
    w = np.ascontiguousarray(np.asarray(inputs["W"], dtype=np.float32))
    maps = []
    for c in range(N_CORES):
        sl = slice(c * B_LOC, (c + 1) * B_LOC)
        maps.append(
            {
                "a_is": a_is[sl],
                "a_em": a_em[sl],
                "m_in": m[sl],
                "tr_m": tr_m[sl],
                "w_in": w,
            }
        )
    return maps


def run(inputs, trace=False, **kw):
    """Run on all 8 cores; returns (full_output, BassKernelResults)."""
    nc = _get_nc()
    res = run_bass_kernel_spmd(
        nc, _in_maps(inputs), list(range(N_CORES)), trace=trace, **kw
    )
    total = float(sum(np.float32(r["out"][0, 0]) for r in res.results))
    return np.array([total], dtype=np.float32), res


def kernel(**inputs) -> np.ndarray:
    out, _ = run(inputs, trace=False)
    return out
